# revision 1
# baseline (speedup 1.0000x reference)
"""ExternalAttention kernel for Trainium2 (8 NeuronCores, data-parallel on batch).

y = relu(x + Wv @ (l1norm_S(softmax_n(Wk @ x))))  per batch, with
x: [16, 512, 64, 64] f32, Wk: [8, 512], Wv: [512, 8].

Sharding: batch 16 -> 2 per core; Wk/Wv replicated. All softmax/L1 stats are
per (batch, s)/(batch, token), so fully local per core.

Matmuls run in fp32r (4x faster than fp32 on the PE for free dim >= 512);
x is rounded to fp32r during the load DMA (gpsimd casting DMA). The
residual add runs ON THE PE: an f32r identity matmul loads x into PSUM
and matmul2 accumulates onto it; ACT's relu drains PSUM back into the x
tile for the store. Result carries only ~1.6e-4-relative fp32r roundings
(~3.5e-4 of output scale). Cost-model (TimelineSim) per-core time:
97.6 us vs a 93.5 us DMA-transfer floor for the 32 MiB/core of traffic.
"""

import numpy as np

import concourse.bass as bass
import concourse.mybir as mybir
import concourse.tile as tile
from concourse import bacc
from concourse.bass_utils import run_bass_kernel_spmd
from concourse.masks import make_identity

F32 = mybir.dt.float32
F32R = mybir.dt.float32r

B, C, HH, WW = 16, 512, 64, 64
N = HH * WW          # 4096 tokens
S = 8                # attention "heads"/keys
NCORES = 8
BLOC = B // NCORES   # 2 batches per core
CCH = 128            # channel chunk == partition dim
NK = C // CCH        # 4 channel chunks
NCOL = 512           # matmul moving free dim (one PSUM bank of f32)
NJ = N // NCOL       # 8 column chunks
HALF = 1024          # x tile width (512 KiB DMA grain)
NH = N // HALF       # 2 halves
JPH = HALF // NCOL   # 4 column chunks per half
EPS = 1e-9


def build_nc(exact_x=False, inplace_out=True):
    nc = bacc.Bacc("TRN2")
    x = nc.dram_tensor("x", [BLOC, C, N], F32, kind="ExternalInput")
    wk = nc.dram_tensor("wk", [S, C], F32, kind="ExternalInput")
    wv = nc.dram_tensor("wv", [C, S], F32, kind="ExternalInput")
    y = nc.dram_tensor("y", [BLOC, C, N], F32, kind="ExternalOutput")

    mult = mybir.AluOpType.mult
    Exp = mybir.ActivationFunctionType.Exp
    Relu = mybir.ActivationFunctionType.Relu

    with tile.TileContext(nc) as tc:
        with (
            tc.tile_pool(name="const", bufs=1) as constp,
            tc.tile_pool(name="xt", bufs=2 * NK * NH) as xp,
            tc.tile_pool(name="u9", bufs=2) as up,
            tc.tile_pool(name="small", bufs=2) as sp,
            tc.tile_pool(name="cols", bufs=3) as cp,
            tc.tile_pool(name="xr", bufs=12) as xrp,
            tc.tile_pool(name="psE", bufs=2, space="PSUM") as psep,
            tc.tile_pool(name="psD", bufs=2, space="PSUM") as psdp,
            tc.tile_pool(name="psY", bufs=4, space="PSUM") as psyp,
        ):
            # --- constants -------------------------------------------------
            # Load weights with CONTIGUOUS descriptors (a transposing gather
            # DMA costs ~3.6us of 4-byte descriptors), transpose on the PE,
            # and round to f32r in the PSUM->SBUF copies.
            ident = constp.tile([CCH, CCH], F32)
            make_identity(nc, ident)
            identR = constp.tile([CCH, CCH], F32R)
            nc.vector.tensor_copy(out=identR, in_=ident)
            # WkT[c, k, s] = Wk[s, 128k + c]; lhsT for matmul1 is WkT[:, k, :]
            wk_sb = constp.tile([S, C], F32)
            nc.sync.dma_start(out=wk_sb, in_=wk[:, :])
            wkT = constp.tile([CCH, NK, S], F32R)
            for k in range(NK):
                pt = psyp.tile([CCH, S], F32, tag="psY")
                nc.tensor.transpose(
                    pt, in_=wk_sb[:, k * CCH:(k + 1) * CCH], identity=ident[0:S, 0:S]
                )
                nc.vector.tensor_copy(out=wkT[:, k, :], in_=pt)
            # WvT[s, c] = Wv[c, s]; lhsT for matmul2 is WvT[:, k*128:...]
            wv_sb = constp.tile([CCH, NK, S], F32)
            for k in range(NK):
                nc.sync.dma_start(
                    out=wv_sb[:, k, :], in_=wv[k * CCH:(k + 1) * CCH, :]
                )
            wvT = constp.tile([S, C], F32R)
            for k in range(NK):
                pt = psyp.tile([S, CCH], F32, tag="psY")
                nc.tensor.transpose(pt, in_=wv_sb[:, k, :], identity=ident)
                nc.vector.tensor_copy(out=wvT[:, k * CCH:(k + 1) * CCH], in_=pt)
            ones8 = constp.tile([S, S], F32)
            nc.vector.memset(ones8, 1.0)
            # K=1 matmul operands that add EPS to every row of the denominator
            # (memset can't emit f32r, so stage f32 then round via copy)
            eps_lhs0 = constp.tile([1, S], F32)
            nc.vector.memset(eps_lhs0, EPS)
            eps_lhs = constp.tile([1, S], F32R)
            nc.vector.tensor_copy(out=eps_lhs, in_=eps_lhs0)
            one_row0 = constp.tile([1, NCOL], F32)
            nc.vector.memset(one_row0, 1.0)
            one_row = constp.tile([1, NCOL], F32R)
            nc.vector.tensor_copy(out=one_row, in_=one_row0)

            for b in range(BLOC):
                # --- load x (cast to fp32r in the DMA) ---------------------
                xt = {}
                for h in range(NH):
                    for k in range(NK):
                        t = xp.tile([CCH, HALF], F32 if exact_x else F32R, tag="xt")
                        nc.gpsimd.dma_start(
                            out=t,
                            in_=x[b, k * CCH:(k + 1) * CCH, h * HALF:(h + 1) * HALF],
                        )
                        xt[k, h] = t

                # --- E = Wk @ x, U = exp(E), Z = sum_n U -------------------
                u9 = up.tile([S, N], F32R, tag="u9")
                zp_t = sp.tile([S, NJ], F32, tag="zp")
                for j in range(NJ):
                    h, jc0 = divmod(j * NCOL, HALF)
                    psE = psep.tile([S, NCOL], F32, tag="psE")
                    for k in range(NK):
                        if exact_x:
                            # JIT-round x to f32r on ACT for the matmul only;
                            # the residual add keeps the exact f32 x
                            xr = xrp.tile([CCH, NCOL], F32R, tag="xr")
                            nc.scalar.copy(out=xr, in_=xt[k, h][:, jc0:jc0 + NCOL])
                            rhs = xr
                        else:
                            rhs = xt[k, h][:, jc0:jc0 + NCOL]
                        nc.tensor.matmul(
                            psE,
                            lhsT=wkT[:, k, :],
                            rhs=rhs,
                            start=(k == 0),
                            stop=(k == NK - 1),
                        )
                    nc.scalar.activation(
                        out=u9[:, j * NCOL:(j + 1) * NCOL],
                        in_=psE,
                        func=Exp,
                        accum_out=zp_t[:, j:j + 1],
                    )

                z_t = sp.tile([S, 1], F32, tag="z")
                nc.vector.reduce_sum(out=z_t, in_=zp_t, axis=mybir.AxisListType.X)
                zinv = sp.tile([S, 1], F32, tag="zinv")
                nc.vector.reciprocal(out=zinv, in_=z_t)

                # zlhs rows = zinv[s] broadcast over 8 cols, so
                #   (zlhs.T @ U)[m, n] = sum_s zinv[s] U[s, n]   for all m,
                # then a K=1 matmul with (eps_lhs, one_row) accumulates +EPS.
                zlhs = sp.tile([S, S], F32R, tag="zlhs")
                nc.vector.tensor_scalar_mul(out=zlhs, in0=ones8, scalar1=zinv)

                for j in range(NJ):
                    jc = slice(j * NCOL, (j + 1) * NCOL)
                    h, jc0 = divmod(j * NCOL, HALF)
                    psD = psdp.tile([S, NCOL], F32, tag="psD")
                    nc.tensor.matmul(psD, lhsT=zlhs, rhs=u9[:, jc], start=True, stop=False)
                    nc.tensor.matmul(psD, lhsT=eps_lhs, rhs=one_row, start=False, stop=True)
                    rD = cp.tile([S, NCOL], F32, tag="rD")
                    nc.vector.reciprocal(out=rD, in_=psD)
                    # a2 = (U * zinv) * (1 / denom)
                    a2 = cp.tile([S, NCOL], F32R, tag="a2")
                    nc.vector.scalar_tensor_tensor(
                        out=a2, in0=u9[:, jc], scalar=zinv, in1=rD, op0=mult, op1=mult
                    )
                    for k in range(NK):
                        psY = psyp.tile([CCH, NCOL], F32, tag="psY")
                        xv = xt[k, h][:, jc0:jc0 + NCOL]
                        if inplace_out and not exact_x:
                            # residual add on the PE: psY = I.T@x + Wv@a2,
                            # then relu drains PSUM back into the x tile.
                            nc.tensor.matmul(
                                psY, lhsT=identR, rhs=xv, start=True, stop=False
                            )
                            nc.tensor.matmul(
                                psY,
                                lhsT=wvT[:, k * CCH:(k + 1) * CCH],
                                rhs=a2,
                                start=False,
                                stop=True,
                            )
                            nc.scalar.activation(out=xv, in_=psY, func=Relu)
                            nc.sync.dma_start(
                                out=y[b, k * CCH:(k + 1) * CCH, jc],
                                in_=xv.bitcast(F32),
                            )
                            continue
                        nc.tensor.matmul(
                            psY,
                            lhsT=wvT[:, k * CCH:(k + 1) * CCH],
                            rhs=a2,
                            start=True,
                            stop=True,
                        )
                        if inplace_out:
                            nc.vector.tensor_add(out=xv, in0=xv, in1=psY)
                            nc.scalar.activation(out=xv, in_=xv, func=Relu)
                            src = xv if exact_x else xv.bitcast(F32)
                        else:
                            # exact-f32 epilogue into a separate column tile:
                            # only x itself carries the f32r load rounding
                            ycol = xrp.tile([CCH, NCOL], F32, tag="ycol")
                            nc.vector.tensor_add(
                                out=ycol, in0=xv if exact_x else xv.bitcast(F32), in1=psY
                            )
                            nc.scalar.activation(out=ycol, in_=ycol, func=Relu)
                            src = ycol
                        nc.sync.dma_start(
                            out=y[b, k * CCH:(k + 1) * CCH, jc],
                            in_=src,
                        )

    nc.finalize()
    return nc


_NC_CACHE = None


def _get_nc():
    global _NC_CACHE
    if _NC_CACHE is None:
        _NC_CACHE = build_nc()
    return _NC_CACHE


def kernel(x, Wk, Wv):
    x = np.ascontiguousarray(np.asarray(x, dtype=np.float32))
    Wk = np.ascontiguousarray(np.asarray(Wk, dtype=np.float32))
    Wv = np.ascontiguousarray(np.asarray(Wv, dtype=np.float32))
    assert x.shape == (B, C, HH, WW), x.shape
    xr = x.reshape(B, C, N)

    nc = _get_nc()
    in_maps = [
        {"x": xr[i * BLOC:(i + 1) * BLOC], "wk": Wk, "wv": Wv}
        for i in range(NCORES)
    ]
    res = run_bass_kernel_spmd(nc, in_maps, list(range(NCORES)))
    out = np.concatenate([res.results[i]["y"] for i in range(NCORES)], axis=0)
    return out.reshape(B, C, HH, WW)



# revision 11
# speedup vs baseline: 1.3410x; 1.3410x over previous
"""ExternalAttention kernel for Trainium2 (8 NeuronCores, data-parallel on batch).

y = relu(x + Wv @ (l1norm_S(softmax_n(Wk @ x))))  per batch, with
x: [16, 512, 64, 64] f32, Wk: [8, 512], Wv: [512, 8].

Sharding: batch 16 -> 2 per core; Wk/Wv replicated (tiny, pre-transposed on
host). All softmax/L1 stats are per (batch, token) / per (batch, s), so fully
local per core.

HBM traffic runs in fp16 (x and y are converted at the host boundary), which
halves the 32 MiB/core f32 traffic to 16 MiB/core; the DMA transfer floor is
~46.6 us/core. fp16 carries ~5e-4 relative rounding -- far inside the 2e-2
gate.

Dataflow per batch:
  - load x[b] as one [128, 4k, n] fp16 SBUF tile (c-chunks side by side)
  - E^T = x^T @ WkT computed per 128-token chunk into PSUM [128 tok, 8]
    (moving dim is S=8, so the whole E costs ~256 PE rows per chunk-group)
  - exp on ACT into u^T [128, 32*8]; Z via DVE chunk-reduce + GPSIMD
    partition_all_reduce; a1 = u^T * zinv; denom = sum_s a1; a2^T = a1 * rdn
    (all DVE ops on [128, 256] tiles -- free-dim cost only)
  - a2^T chunks transposed back to [8, 512] groups on the PE, copied to SBUF
  - psY = I.T @ x + WvT.T @ a2 on the PE (residual add on the PE), relu
    drains PSUM back into the x tile (fp16), stores stream out per n-half
"""

import numpy as np

import concourse.bass as bass
import concourse.bass_isa as bass_isa
import concourse.mybir as mybir
import concourse.tile as tile
from concourse import bacc
from concourse.bass_utils import run_bass_kernel_spmd
from concourse.masks import make_identity

F32 = mybir.dt.float32
F16 = mybir.dt.float16

B, C, HH, WW = 16, 512, 64, 64
N = HH * WW           # 4096 tokens
S = 8                 # attention "heads"/keys
NCORES = 8
BLOC = B // NCORES    # 2 batches per core
CCH = 128             # channel chunk == partition dim
NK = C // CCH         # 4 channel chunks
TCH = 128             # tokens per E^T chunk (PSUM partition dim)
NCH = N // TCH        # 32 chunks per batch
GRP = 512             # tokens per a2/psY group (one PSUM bank of f32)
NG = N // GRP         # 8 groups per batch
CPG = GRP // TCH      # 4 chunks per group
HALF = 2048           # load DMA grain (tokens)
NH = N // HALF        # 2 halves
STQ = 1024            # store DMA grain (tokens)
NSQ = N // STQ        # 4 store chunks
KST = 2               # channel chunks per store DMA

add = mybir.AluOpType.add
mult = mybir.AluOpType.mult
Exp = mybir.ActivationFunctionType.Exp
Relu = mybir.ActivationFunctionType.Relu
X = mybir.AxisListType.X


def build_nc():
    nc = bacc.Bacc("TRN2")
    x = nc.dram_tensor("x", [BLOC, C, N], F16, kind="ExternalInput")
    wkt = nc.dram_tensor("wkt", [C, S], F16, kind="ExternalInput")
    wvt = nc.dram_tensor("wvt", [S, C], F16, kind="ExternalInput")
    y = nc.dram_tensor("y", [BLOC, C, N], F16, kind="ExternalOutput")

    with tile.TileContext(nc) as tc:
        with (
            tc.tile_pool(name="const", bufs=1) as constp,
            tc.tile_pool(name="xt", bufs=BLOC) as xp,
            tc.tile_pool(name="ut", bufs=2 * BLOC) as up,
            tc.tile_pool(name="small", bufs=2 * BLOC) as sp,
            tc.tile_pool(name="a2s", bufs=3) as a2p,
            tc.tile_pool(name="psE", bufs=BLOC, space="PSUM") as psep,
            tc.tile_pool(name="psA", bufs=2, space="PSUM") as psap,
            tc.tile_pool(name="psY", bufs=3, space="PSUM") as psyp,
        ):
            # --- constants -------------------------------------------------
            ident = constp.tile([CCH, CCH], F32)
            make_identity(nc, ident)
            identH = constp.tile([CCH, CCH], F16)
            nc.vector.tensor_copy(out=identH, in_=ident)
            # wkT[c, k, s] = Wk[s, 128k + c] (host passes Wk.T contiguous)
            wkT = constp.tile([CCH, NK, S], F16)
            nc.sync.dma_start(
                out=wkT, in_=wkt[:, :].rearrange("(k c) s -> c k s", k=NK)
            )
            # wvT[s, c] = Wv[c, s] (host passes Wv.T contiguous)
            wvT = constp.tile([S, C], F16)
            nc.sync.dma_start(out=wvT, in_=wvt[:, :])

            for b in range(BLOC):
                # --- load x[b] as [c, k, n] fp16 ---------------------------
                xt = xp.tile([CCH, NK, N], F16, tag="xt")
                for h in range(NH):
                    hs = slice(h * HALF, (h + 1) * HALF)
                    nc.sync.dma_start(
                        out=xt[:, :, hs],
                        in_=x[b, :, hs].rearrange("(k c) n -> c k n", k=NK),
                    )

                # --- E^T per token chunk: psET[tok, j*S+s] -----------------
                psET = psep.tile([CCH, NCH * S], F32, tag="psE")
                for j in range(NCH):
                    js = slice(j * TCH, (j + 1) * TCH)
                    for k in range(NK):
                        nc.tensor.matmul(
                            psET[:, j * S:(j + 1) * S],
                            lhsT=xt[:, k, js],
                            rhs=wkT[:, k, :],
                            start=(k == 0),
                            stop=(k == NK - 1),
                        )

                # --- u^T = exp(E^T); Z; a2^T -------------------------------
                uT = up.tile([CCH, NCH * S], F16, tag="uT")
                halfc = NCH * S // 2
                for h in range(2):
                    cs = slice(h * halfc, (h + 1) * halfc)
                    nc.scalar.activation(out=uT[:, cs], in_=psET[:, cs], func=Exp)
                zsum = sp.tile([CCH, S], F32, tag="zsum")
                nc.vector.reduce_sum(
                    out=zsum, in_=uT.rearrange("p (j s) -> p s j", s=S), axis=X
                )
                zrep = sp.tile([CCH, S], F32, tag="zrep")
                nc.gpsimd.partition_all_reduce(
                    zrep, zsum, channels=CCH, reduce_op=bass_isa.ReduceOp.add
                )
                zinv = sp.tile([CCH, S], F16, tag="zinv")
                with nc.allow_low_precision(reason="fp16 attn weights; 2e-2 gate"):
                    nc.vector.reciprocal(out=zinv, in_=zrep)

                a1 = up.tile([CCH, NCH * S], F16, tag="a1")
                nc.vector.tensor_tensor(
                    out=a1.rearrange("p (j s) -> p j s", s=S),
                    in0=uT.rearrange("p (j s) -> p j s", s=S),
                    in1=zinv[:, None, :].broadcast_to([CCH, NCH, S]),
                    op=mult,
                )
                dn = sp.tile([CCH, NCH], F32, tag="dn")
                nc.vector.reduce_sum(
                    out=dn, in_=a1.rearrange("p (j s) -> p j s", s=S), axis=X
                )
                rdn = sp.tile([CCH, NCH], F16, tag="rdn")
                with nc.allow_low_precision(reason="fp16 attn weights; 2e-2 gate"):
                    nc.vector.reciprocal(out=rdn, in_=dn)
                a2T = up.tile([CCH, NCH * S], F16, tag="a2T")
                nc.vector.tensor_tensor(
                    out=a2T.rearrange("p (j s) -> p j s", s=S),
                    in0=a1.rearrange("p (j s) -> p j s", s=S),
                    in1=rdn[:, :, None].broadcast_to([CCH, NCH, S]),
                    op=mult,
                )

                # --- per group: transpose a2, psY = I.T@x + WvT.T@a2 -------
                for g in range(NG):
                    gs = slice(g * GRP, (g + 1) * GRP)
                    psA2 = psap.tile([S, GRP], F16, tag="psA")
                    for t in range(CPG):
                        j = g * CPG + t
                        nc.tensor.transpose(
                            psA2[:, t * TCH:(t + 1) * TCH],
                            in_=a2T[:, j * S:(j + 1) * S],
                            identity=identH,
                        )
                    a2s = a2p.tile([S, GRP], F16, tag="a2s")
                    nc.vector.tensor_copy(out=a2s, in_=psA2)
                    for k in range(NK):
                        psY = psyp.tile([CCH, GRP], F32, tag="psY")
                        xv = xt[:, k, gs]
                        nc.tensor.matmul(
                            psY, lhsT=identH, rhs=xv, start=True, stop=False
                        )
                        nc.tensor.matmul(
                            psY,
                            lhsT=wvT[:, k * CCH:(k + 1) * CCH],
                            rhs=a2s,
                            start=False,
                            stop=True,
                        )
                        if k % 2 == 0:
                            nc.scalar.activation(out=xv, in_=psY, func=Relu)
                        else:
                            nc.vector.tensor_scalar_max(
                                out=xv, in0=psY, scalar1=0.0
                            )

                # --- store ------------------------------------------------
                for q in range(NSQ):
                    qs = slice(q * STQ, (q + 1) * STQ)
                    for kk in range(NK // KST):
                        ks = slice(kk * KST, (kk + 1) * KST)
                        nc.scalar.dma_start(
                            out=y[b, kk * KST * CCH:(kk + 1) * KST * CCH, qs]
                            .rearrange("(k c) n -> c k n", k=KST),
                            in_=xt[:, ks, qs],
                        )

    nc.finalize()
    return nc


_NC_CACHE = None


def _get_nc():
    global _NC_CACHE
    if _NC_CACHE is None:
        _NC_CACHE = build_nc()
    return _NC_CACHE


def kernel(x, Wk, Wv):
    x = np.asarray(x)
    assert x.shape == (B, C, HH, WW), x.shape
    xr = np.ascontiguousarray(x.reshape(B, C, N).astype(np.float16))
    wkt = np.ascontiguousarray(np.asarray(Wk).T.astype(np.float16))
    wvt = np.ascontiguousarray(np.asarray(Wv).T.astype(np.float16))

    nc = _get_nc()
    in_maps = [
        {"x": xr[i * BLOC:(i + 1) * BLOC], "wkt": wkt, "wvt": wvt}
        for i in range(NCORES)
    ]
    res = run_bass_kernel_spmd(nc, in_maps, list(range(NCORES)))
    out = np.concatenate([res.results[i]["y"] for i in range(NCORES)], axis=0)
    return out.astype(np.float32).reshape(B, C, HH, WW)


# revision 29
# speedup vs baseline: 1.8218x; 1.3585x over previous
"""ExternalAttention kernel for Trainium2 (8 NeuronCores, data-parallel on batch).

y = relu(x + Wv @ (l1norm_S(softmax_n(Wk @ x))))  per batch, with
x: [16, 512, 64, 64] f32, Wk: [8, 512], Wv: [512, 8].

Sharding: batch 16 -> 2 per core; Wk/Wv replicated (tiny, pre-transposed on
host). All softmax/L1 stats are per (batch, token) / per (batch, s), so fully
local per core.

HBM traffic runs in fp16 (x and y are converted at the host boundary), which
halves the 32 MiB/core f32 traffic to 16 MiB/core; the DMA transfer floor is
~46.6 us/core. fp16 carries ~5e-4 relative rounding -- far inside the 2e-2
gate.

Dataflow per batch:
  - load x[b] as one [128, 4k, n] fp16 SBUF tile (c-chunks side by side)
  - E^T = x^T @ WkT computed per 128-token chunk into PSUM [128 tok, 8]
    (moving dim is S=8, so the whole E costs ~256 PE rows per batch)
  - exp on ACT into u^T [128, 32*8]; Z via DVE chunk-reduce + GPSIMD
    partition_all_reduce; a1 = u^T * zinv; denom = sum_s a1; a2^T = a1 * rdn
    (all DVE ops on [128, 256] tiles -- free-dim cost only)
  - a2^T chunks transposed back to [8, 512] groups on the PE, copied to SBUF
  - psY = I.T @ x + WvT.T @ a2 on the PE (residual add on the PE); relu
    drains PSUM back into the x tile (fp16) alternating ACT/DVE; stores
    stream out per (1024-token, 256-channel) block
  - the two batches are software-pipelined: b1's E/stats run in the middle
    of b0's psY stream so b1's matmuls start as soon as its attn is ready
"""

import numpy as np

import concourse.bass as bass
import concourse.bass_isa as bass_isa
import concourse.mybir as mybir
import concourse.tile as tile
from concourse import bacc
from concourse.bass_utils import run_bass_kernel_spmd
from concourse.masks import make_identity

F32 = mybir.dt.float32
F16 = mybir.dt.float16

B, C, HH, WW = 16, 512, 64, 64
N = HH * WW           # 4096 tokens
S = 8                 # attention "heads"/keys
NCORES = 8
BLOC = B // NCORES    # 2 batches per core
CCH = 128             # channel chunk == partition dim
NK = C // CCH         # 4 channel chunks
TCH = 128             # tokens per E^T chunk (PSUM partition dim)
NCH = N // TCH        # 32 chunks per batch
GRP = 512             # tokens per a2/psY group (one PSUM bank of f32)
NG = N // GRP         # 8 groups per batch
CPG = GRP // TCH      # 4 chunks per group
QL = 1024             # load DMA grain (tokens)
NQL = N // QL         # 4 load quarters
STQ = 1024            # store DMA grain (tokens)
NSQ = N // STQ        # 4 store chunks
KST = 2               # channel chunks per store DMA

mult = mybir.AluOpType.mult
Exp = mybir.ActivationFunctionType.Exp
Relu = mybir.ActivationFunctionType.Relu
X = mybir.AxisListType.X


def build_nc():
    nc = bacc.Bacc("TRN2")
    x = nc.dram_tensor("x", [BLOC, C, N], F16, kind="ExternalInput")
    wkt = nc.dram_tensor("wkt", [C, S], F16, kind="ExternalInput")
    wvt = nc.dram_tensor("wvt", [S, C], F16, kind="ExternalInput")
    y = nc.dram_tensor("y", [BLOC, C, N], F16, kind="ExternalOutput")

    with tile.TileContext(nc) as tc:
        with (
            tc.tile_pool(name="const", bufs=1) as constp,
            tc.tile_pool(name="xt", bufs=BLOC) as xp,
            tc.tile_pool(name="ut", bufs=2 * BLOC) as up,
            tc.tile_pool(name="small", bufs=2 * BLOC) as sp,
            tc.tile_pool(name="a2s", bufs=2 * NG) as a2p,
            tc.tile_pool(name="psE", bufs=BLOC, space="PSUM") as psep,
            tc.tile_pool(name="psA", bufs=2, space="PSUM") as psap,
            tc.tile_pool(name="psY", bufs=4, space="PSUM") as psyp,
        ):
            # --- constants (weights on the scalar queue; x loads go first
            # on the sync queue) ------------------------------------------
            ident = constp.tile([CCH, CCH], F32)
            make_identity(nc, ident)
            identH = constp.tile([CCH, CCH], F16)
            nc.vector.tensor_copy(out=identH, in_=ident)
            # wkT[c, k, s] = Wk[s, 128k + c] (host passes Wk.T contiguous)
            wkT = constp.tile([CCH, NK, S], F16)
            nc.scalar.dma_start(
                out=wkT, in_=wkt[:, :].rearrange("(k c) s -> c k s", k=NK)
            )
            # wvT[s, c] = Wv[c, s] (host passes Wv.T contiguous)
            wvT = constp.tile([S, C], F16)
            nc.scalar.dma_start(out=wvT, in_=wvt[:, :])

            # --- all loads upfront --------------------------------------
            xts = []
            for b in range(BLOC):
                xt = xp.tile([CCH, NK, N], F16, tag="xt")
                for h in range(NQL):
                    hs = slice(h * QL, (h + 1) * QL)
                    nc.sync.dma_start(
                        out=xt[:, :, hs],
                        in_=x[b, :, hs].rearrange("(k c) n -> c k n", k=NK),
                    )
                xts.append(xt)

            def e_mat(b, h):
                """E^T for one n-quarter: psET[tok, j*S+s], exp, partial Z."""
                if h == 0:
                    _st[b]["psET"] = psep.tile(
                        [CCH, NCH * S], F32, tag="psE", name="psET"
                    )
                    _st[b]["uT"] = up.tile(
                        [CCH, NCH * S], F16, tag="uT", name="uT"
                    )
                psET = _st[b]["psET"]
                uT = _st[b]["uT"]
                nch_q = NCH // NQL
                jlo, jhi = h * nch_q, (h + 1) * nch_q
                for j in range(jlo, jhi):
                    js = slice(j * TCH, (j + 1) * TCH)
                    for k in range(NK):
                        nc.tensor.matmul(
                            psET[:, j * S:(j + 1) * S],
                            lhsT=xts[b][:, k, js],
                            rhs=wkT[:, k, :],
                            start=(k == 0),
                            stop=(k == NK - 1),
                        )
                cs = slice(jlo * S, jhi * S)
                nc.scalar.activation(out=uT[:, cs], in_=psET[:, cs], func=Exp)
                zsumh = sp.tile([CCH, S], F32, tag=f"zsum{h}", name=f"zsumh{h}")
                nc.vector.reduce_sum(
                    out=zsumh,
                    in_=uT[:, cs].rearrange("p (j s) -> p s j", s=S),
                    axis=X,
                )
                if h == 0:
                    _st[b]["zacc"] = zsumh
                else:
                    zacc = sp.tile([CCH, S], F32, tag=f"zacc{h}", name=f"zacc{h}")
                    nc.vector.tensor_tensor(
                        out=zacc, in0=_st[b]["zacc"], in1=zsumh,
                        op=mybir.AluOpType.add,
                    )
                    _st[b]["zacc"] = zacc

            def stats_z(b):
                """zinv = 1/Z replicated on all partitions."""
                zsum = _st[b]["zacc"]
                zrep = sp.tile([CCH, S], F32, tag="zrep")
                nc.gpsimd.partition_all_reduce(
                    zrep, zsum, channels=CCH, reduce_op=bass_isa.ReduceOp.add
                )
                zinv = sp.tile([CCH, S], F16, tag="zinv")
                with nc.allow_low_precision(reason="fp16 attn; 2e-2 gate"):
                    nc.vector.reciprocal(out=zinv, in_=zrep)
                _st[b]["zinv"] = zinv
                a2T = up.tile([CCH, NCH * S], F16, tag="a2T")
                _st[b]["a2T"] = a2T

            def stats_a2(b, h):
                """a2^T for one n-quarter (chunks [h*8, (h+1)*8))."""
                uT = _st[b]["uT"]
                zinv = _st[b]["zinv"]
                a2T = _st[b]["a2T"]
                nh = NCH // 4
                cs = slice(h * nh * S, (h + 1) * nh * S)
                a1 = up.tile([CCH, nh * S], F16, tag=f"a1{h}", name=f"a1{h}")
                nc.vector.tensor_tensor(
                    out=a1.rearrange("p (j s) -> p j s", s=S),
                    in0=uT[:, cs].rearrange("p (j s) -> p j s", s=S),
                    in1=zinv[:, None, :].broadcast_to([CCH, nh, S]),
                    op=mult,
                )
                dn = sp.tile([CCH, nh], F32, tag=f"dn{h}", name=f"dn{h}")
                nc.vector.reduce_sum(
                    out=dn, in_=a1.rearrange("p (j s) -> p j s", s=S), axis=X
                )
                rdn = sp.tile([CCH, nh], F16, tag=f"rdn{h}", name=f"rdn{h}")
                with nc.allow_low_precision(reason="fp16 attn; 2e-2 gate"):
                    nc.vector.reciprocal(out=rdn, in_=dn)
                nc.vector.tensor_tensor(
                    out=a2T[:, cs].rearrange("p (j s) -> p j s", s=S),
                    in0=a1.rearrange("p (j s) -> p j s", s=S),
                    in1=rdn[:, :, None].broadcast_to([CCH, nh, S]),
                    op=mult,
                )

            def transp(b, groups):
                """a2 groups back to [S, 512] via PE transpose + DVE copy."""
                a2T = _st[b]["a2T"]
                a2gs = _st[b].setdefault("a2gs", {})
                for g in groups:
                    psA2 = psap.tile([S, GRP], F16, tag="psA")
                    for t in range(CPG):
                        j = g * CPG + t
                        nc.tensor.transpose(
                            psA2[:, t * TCH:(t + 1) * TCH],
                            in_=a2T[:, j * S:(j + 1) * S],
                            identity=identH,
                        )
                    a2s = a2p.tile([S, GRP], F16, tag="a2s")
                    nc.vector.tensor_copy(out=a2s, in_=psA2)
                    a2gs[g] = a2s

            def psy_ident(b, groups):
                """Prefill psY with the residual (identity matmul; x only)."""
                psys = _st[b].setdefault("psys", {})
                for g in groups:
                    gs = slice(g * GRP, (g + 1) * GRP)
                    for k in range(NK):
                        psY = psyp.tile([CCH, GRP], F32, tag="psY")
                        nc.tensor.matmul(
                            psY, lhsT=identH, rhs=xts[b][:, k, gs],
                            start=True, stop=False,
                        )
                        psys[g, k] = psY

            def psy_wv(b, groups):
                """Accumulate WvT.T @ a2 onto psY; relu drains back into xt."""
                psys = _st[b].setdefault("psys", {})
                a2gs = _st[b]["a2gs"]
                for g in groups:
                    gs = slice(g * GRP, (g + 1) * GRP)
                    for k in range(NK):
                        psY = psys.pop((g, k))
                        xv = xts[b][:, k, gs]
                        nc.tensor.matmul(
                            psY,
                            lhsT=wvT[:, k * CCH:(k + 1) * CCH],
                            rhs=a2gs[g],
                            start=False,
                            stop=True,
                        )
                        on_act = k in (0, 2) or (k == 3 and g % 2 == 0)
                        if on_act:
                            nc.scalar.activation(out=xv, in_=psY, func=Relu)
                        else:
                            nc.vector.tensor_scalar_max(
                                out=xv, in0=psY, scalar1=0.0
                            )

            def psy(b, groups):
                """Contiguous ident+wv+drain per (g, k)."""
                a2gs = _st[b]["a2gs"]
                for g in groups:
                    gs = slice(g * GRP, (g + 1) * GRP)
                    for k in range(NK):
                        psY = psyp.tile([CCH, GRP], F32, tag="psY")
                        xv = xts[b][:, k, gs]
                        nc.tensor.matmul(
                            psY, lhsT=identH, rhs=xv, start=True, stop=False
                        )
                        nc.tensor.matmul(
                            psY,
                            lhsT=wvT[:, k * CCH:(k + 1) * CCH],
                            rhs=a2gs[g],
                            start=False,
                            stop=True,
                        )
                        on_act = k in (0, 2) or (k == 3 and g % 2 == 0)
                        if on_act:
                            nc.scalar.activation(out=xv, in_=psY, func=Relu)
                        else:
                            nc.vector.tensor_scalar_max(
                                out=xv, in0=psY, scalar1=0.0
                            )

            def stores(b, g_list):
                for g in g_list:
                    gs = slice(g * GRP, (g + 1) * GRP)
                    nc.sync.dma_start(
                        out=y[b, :, gs].rearrange("(k c) n -> c k n", k=NK),
                        in_=xts[b][:, :, gs],
                    )

            # --- software-pipelined emission ------------------------------
            _st = [dict() for _ in range(BLOC)]
            for q in range(NQL):
                e_mat(0, q)
            psy_ident(0, [0])        # residual prefill during the stats gap
            stats_z(0)
            stats_a2(0, 0)
            transp(0, [0, 1])
            psy_wv(0, [0])
            stores(0, [0])
            stats_a2(0, 1)
            psy(0, [1])
            stores(0, [1])
            transp(0, [2, 3])
            stats_a2(0, 2)
            psy(0, [2])
            stores(0, [2])
            e_mat(1, 0)
            stats_a2(0, 3)
            psy(0, [3])
            stores(0, [3])
            transp(0, [4, 5])
            e_mat(1, 1)
            psy(0, [4])
            stores(0, [4])
            e_mat(1, 2)
            e_mat(1, 3)
            stats_z(1)
            stats_a2(1, 0)
            transp(0, [6, 7])
            psy(0, [5])
            stores(0, [5])
            transp(1, [0, 1])
            stats_a2(1, 1)
            psy(0, [6])
            stores(0, [6])
            transp(1, [2, 3])
            stats_a2(1, 2)
            psy(0, [7])
            stores(0, [7])
            stats_a2(1, 3)
            psy(1, [0])
            stores(1, [0])
            transp(1, [4, 5])
            psy(1, [1])
            stores(1, [1])
            psy(1, [2])
            stores(1, [2])
            transp(1, [6, 7])
            psy(1, [3])
            stores(1, [3])
            psy(1, [4])
            stores(1, [4])
            psy(1, [5])
            stores(1, [5])
            psy(1, [6])
            stores(1, [6])
            psy(1, [7])
            # final store split small to shorten the tail
            for part in range(2):
                ps = slice(7 * GRP + part * GRP // 2,
                           7 * GRP + (part + 1) * GRP // 2)
                nc.sync.dma_start(
                    out=y[1, :, ps].rearrange("(k c) n -> c k n", k=NK),
                    in_=xts[1][:, :, ps],
                )

    nc.finalize()
    return nc


_NC_CACHE = None


def _get_nc():
    global _NC_CACHE
    if _NC_CACHE is None:
        _NC_CACHE = build_nc()
    return _NC_CACHE


def kernel(x, Wk, Wv):
    x = np.asarray(x)
    assert x.shape == (B, C, HH, WW), x.shape
    xr = np.ascontiguousarray(x.reshape(B, C, N).astype(np.float16))
    wkt = np.ascontiguousarray(np.asarray(Wk).T.astype(np.float16))
    wvt = np.ascontiguousarray(np.asarray(Wv).T.astype(np.float16))

    nc = _get_nc()
    in_maps = [
        {"x": xr[i * BLOC:(i + 1) * BLOC], "wkt": wkt, "wvt": wvt}
        for i in range(NCORES)
    ]
    res = run_bass_kernel_spmd(nc, in_maps, list(range(NCORES)))
    out = np.concatenate([res.results[i]["y"] for i in range(NCORES)], axis=0)
    return out.astype(np.float32).reshape(B, C, HH, WW)


# revision 38
# speedup vs baseline: 1.8333x; 1.0063x over previous
"""ExternalAttention kernel for Trainium2 (8 NeuronCores, data-parallel on batch).

y = relu(x + Wv @ (l1norm_S(softmax_n(Wk @ x))))  per batch, with
x: [16, 512, 64, 64] f32, Wk: [8, 512], Wv: [512, 8].

Sharding: batch 16 -> 2 per core; Wk/Wv replicated (tiny, pre-transposed on
host). All softmax/L1 stats are per (batch, token) / per (batch, s), so fully
local per core.

HBM traffic runs in fp16 (x and y are converted at the host boundary), which
halves the 32 MiB/core f32 traffic to 16 MiB/core; the DMA transfer floor is
~46.6 us/core. fp16 carries ~5e-4 relative rounding -- far inside the 2e-2
gate.

Dataflow per batch:
  - load x[b] as one [128, 4k, n] fp16 SBUF tile (c-chunks side by side)
  - E^T = x^T @ WkT computed per 128-token chunk into PSUM [128 tok, 8]
    (moving dim is S=8, so the whole E costs ~256 PE rows per batch)
  - exp on ACT into u^T [128, 32*8]; Z via DVE chunk-reduce + GPSIMD
    partition_all_reduce; a1 = u^T * zinv; denom = sum_s a1; a2^T = a1 * rdn
    (all DVE ops on [128, 256] tiles -- free-dim cost only)
  - a2^T chunks transposed back to [8, 512] groups on the PE, copied to SBUF
  - psY = I.T @ x + WvT.T @ a2 on the PE (residual add on the PE); relu
    drains PSUM back into the x tile (fp16) alternating ACT/DVE; stores
    stream out per (1024-token, 256-channel) block
  - the two batches are software-pipelined: b1's E/stats run in the middle
    of b0's psY stream so b1's matmuls start as soon as its attn is ready
"""

import numpy as np

import concourse.bass as bass
import concourse.bass_isa as bass_isa
import concourse.mybir as mybir
import concourse.tile as tile
from concourse import bacc
from concourse.bass_utils import run_bass_kernel_spmd
from concourse.masks import make_identity

F32 = mybir.dt.float32
F16 = mybir.dt.float16

B, C, HH, WW = 16, 512, 64, 64
N = HH * WW           # 4096 tokens
S = 8                 # attention "heads"/keys
NCORES = 8
BLOC = B // NCORES    # 2 batches per core
CCH = 128             # channel chunk == partition dim
NK = C // CCH         # 4 channel chunks
TCH = 128             # tokens per E^T chunk (PSUM partition dim)
NCH = N // TCH        # 32 chunks per batch
GRP = 512             # tokens per a2/psY group (one PSUM bank of f32)
NG = N // GRP         # 8 groups per batch
CPG = GRP // TCH      # 4 chunks per group
QL = 1024             # load DMA grain (tokens)
NQL = N // QL         # 4 load quarters
STQ = 1024            # store DMA grain (tokens)
NSQ = N // STQ        # 4 store chunks
KST = 2               # channel chunks per store DMA

mult = mybir.AluOpType.mult
Exp = mybir.ActivationFunctionType.Exp
Relu = mybir.ActivationFunctionType.Relu
X = mybir.AxisListType.X


def build_nc():
    nc = bacc.Bacc("TRN2")
    x = nc.dram_tensor("x", [BLOC, C, N], F16, kind="ExternalInput")
    wkt = nc.dram_tensor("wkt", [C, S], F16, kind="ExternalInput")
    wvt = nc.dram_tensor("wvt", [S, C], F16, kind="ExternalInput")
    y = nc.dram_tensor("y", [BLOC, C, N], F16, kind="ExternalOutput")

    with tile.TileContext(nc) as tc:
        with (
            tc.tile_pool(name="const", bufs=1) as constp,
            tc.tile_pool(name="xt", bufs=BLOC) as xp,
            tc.tile_pool(name="ut", bufs=2 * BLOC) as up,
            tc.tile_pool(name="small", bufs=2 * BLOC) as sp,
            tc.tile_pool(name="a2s", bufs=2 * NG) as a2p,
            tc.tile_pool(name="psE", bufs=1, space="PSUM") as psep,
            tc.tile_pool(name="psA", bufs=2, space="PSUM") as psap,
            tc.tile_pool(name="psY", bufs=5, space="PSUM") as psyp,
        ):
            # --- constants (weights on the scalar queue; x loads go first
            # on the sync queue) ------------------------------------------
            ident = constp.tile([CCH, CCH], F32)
            make_identity(nc, ident)
            identH = constp.tile([CCH, CCH], F16)
            nc.vector.tensor_copy(out=identH, in_=ident)
            # wkT[c, k, s] = Wk[s, 128k + c] (host passes Wk.T contiguous)
            wkT = constp.tile([CCH, NK, S], F16)
            nc.scalar.dma_start(
                out=wkT, in_=wkt[:, :].rearrange("(k c) s -> c k s", k=NK)
            )
            # wvT[s, c] = Wv[c, s] (host passes Wv.T contiguous)
            wvT = constp.tile([S, C], F16)
            nc.scalar.dma_start(out=wvT, in_=wvt[:, :])

            # --- all loads upfront --------------------------------------
            xts = []
            for b in range(BLOC):
                xt = xp.tile([CCH, NK, N], F16, tag="xt")
                for h in range(NQL):
                    hs = slice(h * QL, (h + 1) * QL)
                    nc.sync.dma_start(
                        out=xt[:, :, hs],
                        in_=x[b, :, hs].rearrange("(k c) n -> c k n", k=NK),
                    )
                xts.append(xt)

            def e_mat(b, h):
                """E^T for one n-quarter: psET[tok, j*S+s], exp, partial Z."""
                if h == 0:
                    _st[b]["psET"] = psep.tile(
                        [CCH, NCH * S], F32, tag="psE", name="psET"
                    )
                    _st[b]["uT"] = up.tile(
                        [CCH, NCH * S], F16, tag="uT", name="uT"
                    )
                psET = _st[b]["psET"]
                uT = _st[b]["uT"]
                nch_q = NCH // NQL
                jlo, jhi = h * nch_q, (h + 1) * nch_q
                for j in range(jlo, jhi):
                    js = slice(j * TCH, (j + 1) * TCH)
                    for k in range(NK):
                        nc.tensor.matmul(
                            psET[:, j * S:(j + 1) * S],
                            lhsT=xts[b][:, k, js],
                            rhs=wkT[:, k, :],
                            start=(k == 0),
                            stop=(k == NK - 1),
                        )
                cs = slice(jlo * S, jhi * S)
                nc.scalar.activation(out=uT[:, cs], in_=psET[:, cs], func=Exp)
                zsumh = sp.tile([CCH, S], F32, tag=f"zsum{h}", name=f"zsumh{h}")
                nc.vector.reduce_sum(
                    out=zsumh,
                    in_=uT[:, cs].rearrange("p (j s) -> p s j", s=S),
                    axis=X,
                )
                if h == 0:
                    _st[b]["zacc"] = zsumh
                else:
                    zacc = sp.tile([CCH, S], F32, tag=f"zacc{h}", name=f"zacc{h}")
                    nc.vector.tensor_tensor(
                        out=zacc, in0=_st[b]["zacc"], in1=zsumh,
                        op=mybir.AluOpType.add,
                    )
                    _st[b]["zacc"] = zacc

            def stats_z(b):
                """zinv = 1/Z replicated on all partitions."""
                zsum = _st[b]["zacc"]
                zrep = sp.tile([CCH, S], F32, tag="zrep")
                nc.gpsimd.partition_all_reduce(
                    zrep, zsum, channels=CCH, reduce_op=bass_isa.ReduceOp.add
                )
                zinv = sp.tile([CCH, S], F16, tag="zinv")
                with nc.allow_low_precision(reason="fp16 attn; 2e-2 gate"):
                    nc.vector.reciprocal(out=zinv, in_=zrep)
                _st[b]["zinv"] = zinv
                a2T = up.tile([CCH, NCH * S], F16, tag="a2T")
                _st[b]["a2T"] = a2T

            def stats_a2(b, h):
                """a2^T for one n-quarter (chunks [h*8, (h+1)*8))."""
                uT = _st[b]["uT"]
                zinv = _st[b]["zinv"]
                a2T = _st[b]["a2T"]
                nh = NCH // 4
                cs = slice(h * nh * S, (h + 1) * nh * S)
                a1 = up.tile([CCH, nh * S], F16, tag=f"a1{h}", name=f"a1{h}")
                nc.vector.tensor_tensor(
                    out=a1.rearrange("p (j s) -> p j s", s=S),
                    in0=uT[:, cs].rearrange("p (j s) -> p j s", s=S),
                    in1=zinv[:, None, :].broadcast_to([CCH, nh, S]),
                    op=mult,
                )
                dn = sp.tile([CCH, nh], F32, tag=f"dn{h}", name=f"dn{h}")
                nc.vector.reduce_sum(
                    out=dn, in_=a1.rearrange("p (j s) -> p j s", s=S), axis=X
                )
                rdn = sp.tile([CCH, nh], F16, tag=f"rdn{h}", name=f"rdn{h}")
                with nc.allow_low_precision(reason="fp16 attn; 2e-2 gate"):
                    nc.vector.reciprocal(out=rdn, in_=dn)
                nc.vector.tensor_tensor(
                    out=a2T[:, cs].rearrange("p (j s) -> p j s", s=S),
                    in0=a1.rearrange("p (j s) -> p j s", s=S),
                    in1=rdn[:, :, None].broadcast_to([CCH, nh, S]),
                    op=mult,
                )

            def transp(b, groups):
                """a2 groups back to [S, 512] via PE transpose + DVE copy."""
                a2T = _st[b]["a2T"]
                a2gs = _st[b].setdefault("a2gs", {})
                for g in groups:
                    psA2 = psap.tile([S, GRP], F16, tag="psA")
                    for t in range(CPG):
                        j = g * CPG + t
                        nc.tensor.transpose(
                            psA2[:, t * TCH:(t + 1) * TCH],
                            in_=a2T[:, j * S:(j + 1) * S],
                            identity=identH,
                        )
                    a2s = a2p.tile([S, GRP], F16, tag="a2s")
                    nc.vector.tensor_copy(out=a2s, in_=psA2)
                    a2gs[g] = a2s

            def psy_ident(b, groups):
                """Prefill psY with the residual (identity matmul; x only)."""
                psys = _st[b].setdefault("psys", {})
                for g in groups:
                    gs = slice(g * GRP, (g + 1) * GRP)
                    for k in range(NK):
                        psY = psyp.tile([CCH, GRP], F32, tag="psY")
                        nc.tensor.matmul(
                            psY, lhsT=identH, rhs=xts[b][:, k, gs],
                            start=True, stop=False,
                        )
                        psys[g, k] = psY

            def psy_wv(b, groups):
                """Accumulate WvT.T @ a2 onto psY; relu drains back into xt."""
                psys = _st[b].setdefault("psys", {})
                a2gs = _st[b]["a2gs"]
                for g in groups:
                    gs = slice(g * GRP, (g + 1) * GRP)
                    for k in range(NK):
                        psY = psys.pop((g, k))
                        xv = xts[b][:, k, gs]
                        nc.tensor.matmul(
                            psY,
                            lhsT=wvT[:, k * CCH:(k + 1) * CCH],
                            rhs=a2gs[g],
                            start=False,
                            stop=True,
                        )
                        on_act = k in (0, 2) or (k == 3 and g % 2 == 0)
                        if on_act:
                            nc.scalar.activation(out=xv, in_=psY, func=Relu)
                        else:
                            nc.vector.tensor_scalar_max(
                                out=xv, in0=psY, scalar1=0.0
                            )

            # residual add moved PE -> DVE for these tiles (PE is the
            # critical path; DVE has slack in the b0 backlog region)
            OFFLOAD = set()

            def psy(b, groups):
                """Contiguous ident+wv+drain per (g, k)."""
                a2gs = _st[b]["a2gs"]
                for g in groups:
                    gs = slice(g * GRP, (g + 1) * GRP)
                    for k in range(NK):
                        psY = psyp.tile([CCH, GRP], F32, tag="psY")
                        xv = xts[b][:, k, gs]
                        off = (b, g, k) in OFFLOAD
                        if not off:
                            nc.tensor.matmul(
                                psY, lhsT=identH, rhs=xv, start=True, stop=False
                            )
                        nc.tensor.matmul(
                            psY,
                            lhsT=wvT[:, k * CCH:(k + 1) * CCH],
                            rhs=a2gs[g],
                            start=off,
                            stop=True,
                        )
                        if off:
                            nc.vector.tensor_tensor(
                                out=psY, in0=psY, in1=xv, op=mybir.AluOpType.add
                            )
                        on_act = k in (0, 2) or (k == 3 and g % 2 == 0)
                        if on_act:
                            nc.scalar.activation(out=xv, in_=psY, func=Relu)
                        else:
                            nc.vector.tensor_scalar_max(
                                out=xv, in0=psY, scalar1=0.0
                            )

            def stores(b, g_list):
                eng = nc.sync
                for g in g_list:
                    gs = slice(g * GRP, (g + 1) * GRP)
                    eng.dma_start(
                        out=y[b, :, gs].rearrange("(k c) n -> c k n", k=NK),
                        in_=xts[b][:, :, gs],
                    )

            # --- software-pipelined emission ------------------------------
            _st = [dict() for _ in range(BLOC)]
            for q in range(NQL):
                e_mat(0, q)
            psy_ident(0, [0])        # residual prefill during the stats gap
            stats_z(0)
            stats_a2(0, 0)
            transp(0, [0, 1])
            stats_a2(0, 1)
            transp(0, [2, 3])
            psy_wv(0, [0])
            stores(0, [0])
            psy(0, [1])
            stores(0, [1])
            stats_a2(0, 2)
            psy(0, [2])
            stores(0, [2])
            e_mat(1, 0)
            stats_a2(0, 3)
            psy(0, [3])
            stores(0, [3])
            transp(0, [4, 5])
            e_mat(1, 1)
            psy(0, [4])
            stores(0, [4])
            e_mat(1, 2)
            e_mat(1, 3)
            stats_z(1)
            stats_a2(1, 0)
            transp(0, [6, 7])
            psy(0, [5])
            stores(0, [5])
            transp(1, [0, 1])
            stats_a2(1, 1)
            psy(0, [6])
            stores(0, [6])
            transp(1, [2, 3])
            stats_a2(1, 2)
            psy(0, [7])
            stores(0, [7])
            stats_a2(1, 3)
            psy(1, [0])
            stores(1, [0])
            transp(1, [4, 5])
            psy(1, [1])
            stores(1, [1])
            psy(1, [2])
            stores(1, [2])
            transp(1, [6, 7])
            psy(1, [3])
            stores(1, [3])
            psy(1, [4])
            stores(1, [4])
            psy(1, [5])
            stores(1, [5])
            psy(1, [6])
            stores(1, [6])
            psy(1, [7])
            # final store split small to shorten the tail
            for part in range(2):
                ps = slice(7 * GRP + part * GRP // 2,
                           7 * GRP + (part + 1) * GRP // 2)
                nc.sync.dma_start(
                    out=y[1, :, ps].rearrange("(k c) n -> c k n", k=NK),
                    in_=xts[1][:, :, ps],
                )

    nc.finalize()
    return nc


_NC_CACHE = None


def _get_nc():
    global _NC_CACHE
    if _NC_CACHE is None:
        _NC_CACHE = build_nc()
    return _NC_CACHE


def kernel(x, Wk, Wv):
    x = np.asarray(x)
    assert x.shape == (B, C, HH, WW), x.shape
    xr = np.ascontiguousarray(x.reshape(B, C, N).astype(np.float16))
    wkt = np.ascontiguousarray(np.asarray(Wk).T.astype(np.float16))
    wvt = np.ascontiguousarray(np.asarray(Wv).T.astype(np.float16))

    nc = _get_nc()
    in_maps = [
        {"x": xr[i * BLOC:(i + 1) * BLOC], "wkt": wkt, "wvt": wvt}
        for i in range(NCORES)
    ]
    res = run_bass_kernel_spmd(nc, in_maps, list(range(NCORES)))
    out = np.concatenate([res.results[i]["y"] for i in range(NCORES)], axis=0)
    return out.astype(np.float32).reshape(B, C, HH, WW)
